# revision 66
# baseline (speedup 1.0000x reference)
"""Trainium2 Bass kernel for block-causal attention (nn_Attn_63367947485870).

Sharding: tensor-parallel over heads x data-parallel over batch.
Core c (0..7): batch = c//4, heads = [4g..4g+3] where g = c%4.
Each core computes QKV projection for its 4 heads, RMSNorm+RoPE, block-causal
attention, and a partial out-projection [S, DM] (bf16); the host sums the 4
partials per batch and adds the output bias.

On-device layouts (to keep the contraction dim on partitions):
  - x is passed pre-transposed per batch: xt [DM, S] (bf16)
  - q/k are computed transposed [feat, tok] in head-pair tiles [128, S]; the
    per-head feature order is PERM64 so the RoPE rotate-half is a single DVE
    stream-shuffle (partners 16 apart within each 32-lane quadrant; the sign
    and rope angles are folded into host-built cos/sin tables, and q_scale/
    k_scale into a per-partition scalar of the final rope multiply)
  - the QKV projection runs in two rounds of 8 concurrent PSUM-bank
    accumulators, emitted contraction-major so each arriving xt tile feeds 8
    matmuls (PE stays busy during the input DMA phase)
  - rmsnorm 1/sqrt(ms) values are broadcast across partitions via a
    DRAM-round-trip DMA with a zero-stride partition access pattern
  - v is computed in natural layout [tok, feat] (xT tiles as stationary lhsT)
  - scores are computed transposed [tk, tq]; softmax denominator comes from a
    ones-column appended to the PV lhsT (out row 64); no max-subtraction is
    needed because RMSNorm bounds |q.k|/8 <= 8. Denominator reciprocals take
    the same DRAM-round-trip broadcast; the half1 normalize multiply writes
    partitions 64:128 directly via a partition-offset read of the PV psum.
  - rope for chunk qc+1 is software-pipelined between attention(qc) and the
    out-projection of qc.
"""

import sys

sys.path.insert(0, "/opt/trn_rl_repo")

import numpy as np

import concourse.bass as bass
import concourse.tile as tile
from concourse import mybir
from concourse import bass_utils
import bass_rust

B, S, DM = 2, 2048, 1024
H, HD = 16, 64
TPF = 128
NF = S // TPF  # 16 frames == 16 token tiles
EPS = 1e-6
ROPE_THETA = 10000.0

TC = 512          # token chunk (matmul moving dim)
NTC = S // TC     # 4
NTT = S // 128    # 16 token tiles
ND = DM // 128    # 8 contraction tiles

F32 = mybir.dt.float32
BF16 = mybir.dt.bfloat16
NPBF16 = mybir.dt.np(mybir.dt.bfloat16)
# q/k feature storage order: rope partners 16 apart within each 32-quadrant,
# so the rotate-half is a single DVE stream-shuffle (mask below)
PERM64 = list(range(0, 16)) + list(range(32, 48)) + list(range(16, 32)) + list(range(48, 64))
SHUF_MASK = [(i + 16) % 32 for i in range(32)]

AF = mybir.ActivationFunctionType
OP = mybir.AluOpType


def _split_ctrl_waits(nc, max_waits=1):
    """Walrus in this container rejects instructions carrying more than one
    semaphore wait. Split extras into preceding single-wait NoOps on the same
    engine (queue order preserves the wait-before-execute semantics)."""
    for f in nc.m.functions:
        for blk in f.blocks:
            il = blk.instructions
            i = 0
            while i < len(il):
                inst = il[i]
                si = inst.sync_info
                waits = list(si.on_wait) if si and si.on_wait else []
                if len(waits) > max_waits:
                    keep = waits[:max_waits]
                    extra = waits[max_waits:]
                    pre = [
                        mybir.InstNoOp(
                            name=f"{inst.name}-wsplit{j}",
                            engine=inst.engine,
                            sync_info=bass_rust.SyncInfo(on_wait=[w], on_update=[]),
                        )
                        for j, w in enumerate(extra)
                    ]
                    si.on_wait = keep
                    for j, d in enumerate(pre):
                        il.insert(i + j, d)
                    i += len(pre)
                i += 1


def _build_program():
    nc = bass.Bass("TRN2", target_bir_lowering=False, debug=False)

    xt = nc.dram_tensor("xt", [DM, S], BF16, kind="ExternalInput")
    wqk = nc.dram_tensor("wqk", [DM, 512], BF16, kind="ExternalInput")
    wv = nc.dram_tensor("wv", [DM, 256], BF16, kind="ExternalInput")
    wo = nc.dram_tensor("wo", [256, DM], BF16, kind="ExternalInput")
    bqk = nc.dram_tensor("bqk", [128, 4], F32, kind="ExternalInput")
    bv = nc.dram_tensor("bv", [1, 256], F32, kind="ExternalInput")
    csb = nc.dram_tensor("csb", [64, S], BF16, kind="ExternalInput")
    snb = nc.dram_tensor("snb", [64, S], BF16, kind="ExternalInput")
    qks = nc.dram_tensor("qks", [128, 2], F32, kind="ExternalInput")
    ones2 = nc.dram_tensor("ones2", [128, 2], BF16, kind="ExternalInput")
    outp = nc.dram_tensor("outp", [S, DM], BF16, kind="ExternalOutput")
    # DRAM scratch for partition-broadcast round-trips
    rbscr = nc.dram_tensor("rbscr", [4, 2, S], BF16, kind="Internal")

    with tile.TileContext(nc) as tc:
        with (
            tc.tile_pool(name="const", bufs=1) as cpool,
            tc.tile_pool(name="big", bufs=1) as bpool,
            tc.tile_pool(name="tmp", bufs=6) as tpool,
            tc.tile_pool(name="attn", bufs=6) as apool,
            tc.tile_pool(name="outs", bufs=6) as opool,
            tc.tile_pool(name="psum", bufs=1, space="PSUM") as pspool,
        ):
            # ---- constant / input loads ----
            xts = []
            wqk_sb = []
            wv_sb = []
            for d in range(ND):
                t = cpool.tile([128, 512], BF16, tag=f"wqk{d}", name=f"wqk{d}")
                nc.sync.dma_start(out=t, in_=wqk[d * 128 : (d + 1) * 128, :])
                wqk_sb.append(t)
                t = bpool.tile([128, S], BF16, tag=f"xt{d}", name=f"xt{d}")
                nc.sync.dma_start(out=t, in_=xt[d * 128 : (d + 1) * 128, :])
                xts.append(t)
            for d in range(ND):
                t = cpool.tile([128, 256], BF16, tag=f"wv{d}", name=f"wv{d}")
                nc.sync.dma_start(out=t, in_=wv[d * 128 : (d + 1) * 128, :])
                wv_sb.append(t)
            wo_sb = []
            for p in range(2):
                t = cpool.tile([128, DM], BF16, tag=f"wo{p}", name=f"wo{p}")
                nc.sync.dma_start(out=t, in_=wo[p * 128 : (p + 1) * 128, :])
                wo_sb.append(t)
            bqk_sb = cpool.tile([128, 4], F32, tag="bqk", name="bqk")
            nc.sync.dma_start(out=bqk_sb, in_=bqk[:, :])
            bv_sb = cpool.tile([128, 256], F32, tag="bv", name="bv")
            bv_ap = bv[:, :]
            bv_bcast = bass.AP(
                tensor=bv_ap.tensor, offset=bv_ap.offset, ap=[[0, 128], bv_ap.ap[1]]
            )
            nc.gpsimd.dma_start(out=bv_sb, in_=bv_bcast)
            cs_sb = cpool.tile([128, S], BF16, tag="cs", name="cs")
            sn_sb = cpool.tile([128, S], BF16, tag="sn", name="sn")
            for base_t, t in ((csb, cs_sb), (snb, sn_sb)):
                bap = base_t[:, :]
                rep = bass.AP(
                    tensor=bap.tensor, offset=bap.offset,
                    ap=[[0, 2], [S, 64], [1, S]],
                )
                nc.sync.dma_start(out=t, in_=rep)
            qks_sb = cpool.tile([128, 2], F32, tag="qks", name="qks")
            nc.sync.dma_start(out=qks_sb, in_=qks[:, :])
            ones2_sb = cpool.tile([128, 2], BF16, tag="ones2", name="ones2")
            nc.sync.dma_start(out=ones2_sb, in_=ones2[:, :])
            eps_sb = cpool.tile([2, 1], F32, tag="eps", name="eps")
            nc.vector.memset(eps_sb, EPS)

            # persistent activation tiles
            qb = [
                [bpool.tile([128, TC], BF16, tag=f"qb{p}_{c}", name=f"qb{p}_{c}") for c in range(NTC)]
                for p in range(2)
            ]
            kb = [
                [bpool.tile([128, TC], BF16, tag=f"kb{p}_{c}", name=f"kb{p}_{c}") for c in range(NTC)]
                for p in range(2)
            ]
            qh, kh = qb, kb  # rope output written in place
            vaug = [bpool.tile([128, 4, 65], BF16, tag=f"va{t_}", name=f"va{t_}") for t_ in range(NTT)]
            otp = [
                [bpool.tile([128, TC], BF16, tag=f"otp{p}_{c}", name=f"otp{p}_{c}") for c in range(NTC)]
                for p in range(2)
            ]

            for t_ in range(NTT):
                nc.vector.memset(vaug[t_][:, :, 64:65], 1.0)

            def emit_qkproj_round(r):
                # 8 concurrent accumulators (all 8 PSUM banks), d-major
                # emission so each arriving xt tile feeds 8 matmuls
                fts = (0, 2) if r == 0 else (1, 3)
                sA = pspool.tile([128, 2 * TC], F32, tag="s", name="pjsA", bufs=2)
                sB = pspool.tile([128, 2 * TC], F32, tag="s", name="pjsB", bufs=2)
                t_mm = pspool.tile([128, TC], F32, tag="mm", name="pjmm", bufs=1)
                t_rms = pspool.tile([128, TC], F32, tag="rms", name="pjrms", bufs=1)
                t_oa = pspool.tile([128, TC], F32, tag="oA", name="pjoa", bufs=1)
                t_ob = pspool.tile([128, TC], F32, tag="oB", name="pjob", bufs=1)
                regions = {
                    (0, 0): sA[:, 0:TC], (0, 1): sB[:, 0:TC],
                    (0, 2): t_mm, (0, 3): t_oa,
                    (1, 0): sA[:, TC : 2 * TC], (1, 1): sB[:, TC : 2 * TC],
                    (1, 2): t_rms, (1, 3): t_ob,
                }
                for d in range(ND):
                    for fi, ft in enumerate(fts):
                        for c in range(NTC):
                            nc.tensor.matmul(
                                regions[(fi, c)],
                                wqk_sb[d][:, ft * 128 : (ft + 1) * 128],
                                xts[d][:, c * TC : (c + 1) * TC],
                                start=(d == 0),
                                stop=(d == ND - 1),
                                skip_group_check=True,
                            )
                # evac c-major so each PSUM slot (holding both fi-halves of
                # one c) frees as early as possible for the next round
                for c in range(NTC):
                    for fi, ft in enumerate(fts):
                        dst = qb[ft] if ft < 2 else kb[ft - 2]
                        nc.scalar.activation(
                            dst[c], regions[(fi, c)], AF.Identity,
                            bias=bqk_sb[:, ft : ft + 1],
                        )

            def emit_vnat(t_):
                ps = pspool.tile([128, 256], F32, tag="mm", name="vproj", bufs=1)
                for d in range(ND):
                    nc.tensor.matmul(
                        ps,
                        xts[d][:, t_ * 128 : (t_ + 1) * 128],
                        wv_sb[d],
                        start=(d == 0),
                        stop=(d == ND - 1),
                    )
                nc.vector.tensor_add(
                    vaug[t_][:, :, 0:64],
                    ps.rearrange("p (h e) -> p h e", h=4),
                    bv_sb.rearrange("p (h e) -> p h e", h=4),
                )

            rope_pre = {}

            def emit_rope_prefix(pr, qk, c):
                # DVE-only head of the rope chain; emitted early to fill DVE
                # while the PE runs dense projection matmuls
                src_t = (qb if qk == 0 else kb)[pr]
                sl = slice(c * TC, (c + 1) * TC)
                q2 = tpool.tile([128, TC], BF16, tag="q2", name="q2")
                nc.vector.tensor_mul(q2, src_t[c], src_t[c])
                rot = tpool.tile([128, TC], BF16, tag="rot", name="rot")
                nc.vector.stream_shuffle(rot, src_t[c], SHUF_MASK)
                m1 = tpool.tile([128, TC], BF16, tag="qc", name="qc")
                nc.vector.tensor_mul(m1, src_t[c], cs_sb[:, sl])
                rope_pre[(pr, qk, c)] = (q2, rot, m1)

            def emit_ropenorm_chunk(pr, qk, c):
                src_t = (qb if qk == 0 else kb)[pr]
                dst = (qh if qk == 0 else kh)[pr]
                scri = qk * 2 + pr
                sl = slice(c * TC, (c + 1) * TC)
                if (pr, qk, c) in rope_pre:
                    q2, rot, m1 = rope_pre.pop((pr, qk, c))
                else:
                    q2 = tpool.tile([128, TC], BF16, tag="q2", name="q2")
                    nc.vector.tensor_mul(q2, src_t[c], src_t[c])
                    rot = tpool.tile([128, TC], BF16, tag="rot", name="rot")
                    nc.vector.stream_shuffle(rot, src_t[c], SHUF_MASK)
                    m1 = tpool.tile([128, TC], BF16, tag="qc", name="qc")
                    nc.vector.tensor_mul(m1, src_t[c], cs_sb[:, sl])
                psr = pspool.tile([2, TC], F32, tag="rms", name="rms", bufs=1)
                nc.tensor.matmul(psr, ones2_sb, q2, start=True, stop=True)
                rr = tpool.tile([2, TC], F32, tag="rr", name="rr", bufs=2)
                nc.scalar.activation(
                    rr, psr, AF.Sqrt, bias=eps_sb[:, 0:1], scale=1.0 / HD
                )
                rb = tpool.tile([2, TC], BF16, tag="rb", name="rb", bufs=2)
                with nc.allow_low_precision("bf16 rhs for rhat broadcast"):
                    nc.vector.reciprocal(rb, rr)
                nc.sync.dma_start(out=rbscr[scri, :, sl], in_=rb)
                # partition-broadcast back: rows 0-63 <- head a, 64-127 <- b
                src_ap = rbscr[scri, :, sl]
                bc_ap = bass.AP(
                    tensor=src_ap.tensor,
                    offset=src_ap.offset,
                    ap=[[S, 2], [0, 64], [1, TC]],
                )
                rbb = tpool.tile([128, TC], BF16, tag="rbb", name="rbb", bufs=3)
                nc.sync.dma_start(out=rbb, in_=bc_ap)
                m2 = tpool.tile([128, TC], BF16, tag="qs", name="qs")
                nc.vector.tensor_mul(m2, rot, sn_sb[:, sl])
                m3 = tpool.tile([128, TC], BF16, tag="m3", name="m3")
                nc.gpsimd.tensor_add(m3, m1, m2)
                nc.vector.scalar_tensor_tensor(
                    dst[c], m3, qks_sb[:, qk : qk + 1], rbb, OP.mult, OP.mult
                )

            # ladder: all projections up front (dense PE), then per query
            # chunk qc: v tiles, rope chunk qc of each pair, attention
            emit_qkproj_round(0)
            emit_qkproj_round(1)
            for pr in range(2):
                for qk in range(2):
                    emit_ropenorm_chunk(pr, qk, 0)
            for qc in range(NTC):
                ntk = 4 * qc + 4
                att_tiles = {}

                def emit_scores(pr, tkp):
                    pair = (tkp, tkp + 1)
                    voffs = [max(0, tk - 4 * qc) * 128 for tk in pair]
                    for half, (off0, off1) in enumerate(((0, 64), (64, 128))):
                        sps2 = pspool.tile(
                            [128, 2 * TC], F32, tag="s", name=f"s{half}", bufs=2
                        )
                        at2 = apool.tile(
                            [128, 2 * TC], BF16, tag=f"at{half}", name=f"at{half}", bufs=8
                        )
                        for j, tk in enumerate(pair):
                            voff = voffs[j]
                            ktile = kh[pr][tk // 4]
                            qtile = qh[pr][qc]
                            tko = (tk % 4) * 128
                            quirk = qc == 3 and tk == 0
                            qhi = 384 if quirk else TC
                            nc.tensor.matmul(
                                sps2[:, j * TC + voff : j * TC + qhi],
                                ktile[off0:off1, tko : tko + 128],
                                qtile[off0:off1, voff:qhi],
                                start=True,
                                stop=True,
                            )
                        quirk0 = qc == 3 and tkp == 0
                        if voffs[0] == 0 and voffs[1] == 0 and not quirk0:
                            nc.scalar.activation(at2, sps2, AF.Exp, scale=HD**-0.5)
                        else:
                            for j, tk in enumerate(pair):
                                voff = voffs[j]
                                qhi = 384 if (qc == 3 and tk == 0) else TC
                                nc.scalar.activation(
                                    at2[:, j * TC + voff : j * TC + qhi],
                                    sps2[:, j * TC + voff : j * TC + qhi],
                                    AF.Exp,
                                    scale=HD**-0.5,
                                )
                                if qhi != TC:
                                    nc.vector.memset(
                                        at2[:, j * TC + qhi : (j + 1) * TC], 0.0
                                    )
                        att_tiles[(pr, tkp, half)] = at2

                def emit_pv_pair(pr, oA, oB, tkp):
                    for tk in (tkp, tkp + 1):
                        voff = max(0, tk - 4 * qc) * 128
                        j = tk % 2
                        for half, ps_out_tile in enumerate((oA, oB)):
                            h = 2 * pr + half
                            at2 = att_tiles[(pr, tk - j, half)]
                            nc.tensor.matmul(
                                ps_out_tile[:, voff:TC],
                                vaug[tk][:, h, :],
                                at2[:, j * TC + voff : (j + 1) * TC],
                                start=(tk == 0),
                                stop=(tk == ntk - 1),
                            )

                def emit_normalize(pr, oA, oB):
                    # denominator reciprocals seeded onto the four quadrant
                    # leader partitions (0,32: head a; 64,96: head b), then one
                    # stream-shuffle with mask=[0]*32 replicates each leader
                    # across its 32-lane quadrant -- no DMA round trip. The
                    # half1 mul writes rows 64:128 via an offset read of oB.
                    dnq = tpool.tile([128, TC], BF16, tag="dnq", name="dnq")
                    with nc.allow_low_precision("bf16 denom reciprocal"):
                        nc.vector.reciprocal(dnq[0:1, :], oA[64:65, :])
                        nc.vector.reciprocal(dnq[64:65, :], oB[64:65, :])
                    nc.vector.tensor_copy(dnq[32:33, :], dnq[0:1, :])
                    nc.vector.tensor_copy(dnq[96:97, :], dnq[64:65, :])
                    rcpb = tpool.tile([128, TC], BF16, tag="rcpb", name="rcpb", bufs=3)
                    nc.vector.stream_shuffle(rcpb, dnq, [0] * 32)
                    nc.vector.tensor_mul(
                        otp[pr][qc][0:64, :], oA[0:64, :], rcpb[0:64, :]
                    )
                    nc.vector.tensor_mul(
                        otp[pr][qc][64:128, :], oB[0:64, :], rcpb[64:128, :]
                    )

                pairs = list(range(0, ntk, 2))
                for tkp in pairs:
                    emit_scores(0, tkp)
                # v tiles for this qc: emitted after scores(pr0) so the score
                # matmuls aren't head-of-line blocked behind vnat's PSUM slot
                for t_ in range(4 * qc, 4 * qc + 4):
                    emit_vnat(t_)
                # PV(pr0) interleaved pairwise with scores(pr1): ACT cooks
                # pr1's exps while PE accumulates pr0's PV
                oA0 = pspool.tile([65, TC], F32, tag="oA", name="oA", bufs=1)
                oB0 = pspool.tile([65, TC], F32, tag="oB", name="oB", bufs=1)
                for tkp in pairs:
                    emit_pv_pair(0, oA0, oB0, tkp)
                    emit_scores(1, tkp)
                emit_normalize(0, oA0, oB0)
                oA1 = pspool.tile([65, TC], F32, tag="oA", name="oA", bufs=1)
                oB1 = pspool.tile([65, TC], F32, tag="oB", name="oB", bufs=1)
                for tkp in pairs:
                    emit_pv_pair(1, oA1, oB1, tkp)
                emit_normalize(1, oA1, oB1)
                if qc + 1 < NTC:
                    for spr in range(2):
                        emit_ropenorm_chunk(spr, 0, qc + 1)
                        emit_ropenorm_chunk(spr, 1, qc + 1)
                # out-projection for this qc's 4 query tiles; the last qc
                # can also use the freed oA/oB banks (no later attention)
                for qt in range(4 * qc, 4 * qc + 4):
                    tsl = slice(qt * 128, (qt + 1) * 128)
                    osl = slice((qt % 4) * 128, (qt % 4 + 1) * 128)
                    if qc == NTC - 1 and qt % 2 == 1:
                        tg0, tg1 = "oA", "oB"
                    else:
                        tg0, tg1 = "mm", "rms"
                    po0 = pspool.tile([128, 512], F32, tag=tg0, name="po0", bufs=1)
                    po1 = pspool.tile([128, 512], F32, tag=tg1, name="po1", bufs=1)
                    for p in range(2):
                        nc.tensor.matmul(
                            po0, otp[p][qc][:, osl], wo_sb[p][:, 0:512],
                            start=(p == 0), stop=(p == 1),
                        )
                        nc.tensor.matmul(
                            po1, otp[p][qc][:, osl], wo_sb[p][:, 512:1024],
                            start=(p == 0), stop=(p == 1),
                        )
                    ob = opool.tile([128, DM], BF16, tag="ob", name="ob")
                    nc.vector.tensor_copy(ob[:, 0:512], po0)
                    if qc == NTC - 1:
                        nc.scalar.activation(ob[:, 512:1024], po1, AF.Copy)
                    else:
                        nc.vector.tensor_copy(ob[:, 512:1024], po1)
                    nc.sync.dma_start(out=outp[tsl, :], in_=ob)

    _split_ctrl_waits(nc)
    return nc


_PROGRAM = None


def _get_program():
    global _PROGRAM
    if _PROGRAM is None:
        _PROGRAM = _build_program()
    return _PROGRAM


def _host_inputs(x, Wqkv, bqkv, q_scale, k_scale, Wout, bout):
    """Build the 8 per-core input maps."""
    inv_freq = 1.0 / (ROPE_THETA ** (np.arange(0, HD, 2, dtype=np.float64) / HD))
    pos = np.arange(S, dtype=np.float64)
    ang = pos[None, :] * inv_freq[:, None]  # [32, S]
    cos64 = np.concatenate([np.cos(ang), np.cos(ang)], axis=0)  # [64, S]
    sin64 = np.concatenate([np.sin(ang), np.sin(ang)], axis=0)
    perm = np.asarray(PERM64)
    # storage row j holds original feature perm[j]; rope partner is the
    # SHUF_MASK stream-shuffle; sign is - for the first 16 rows of each
    # 32-quadrant (those hold first-half features)
    sign = np.where(np.arange(64) % 32 < 16, -1.0, 1.0)[:, None]
    csb_np = cos64[perm].astype(NPBF16)          # [64, S], shared q/k
    snb_np = (sin64[perm] * sign).astype(NPBF16)
    qsp = np.asarray(q_scale, np.float64)[perm]
    ksp = np.asarray(k_scale, np.float64)[perm]
    qks_np = np.stack(
        [np.tile(qsp, 2), np.tile(ksp, 2)], axis=1
    ).astype(np.float32)                          # [128, 2]

    Wq = Wqkv[0:DM]          # [1024, 1024] rows = head h dims
    Wk = Wqkv[DM : 2 * DM]
    Wv = Wqkv[2 * DM : 3 * DM]
    bq, bk, bv_full = bqkv[0:DM], bqkv[DM : 2 * DM], bqkv[2 * DM : 3 * DM]

    ones2_np = np.zeros((128, 2), dtype=np.float32)
    ones2_np[0:64, 0] = 1.0
    ones2_np[64:128, 1] = 1.0
    ones2_np = ones2_np.astype(NPBF16)
    perm_l = list(perm)
    in_maps = []
    for c in range(8):
        b, g = divmod(c, 4)
        heads = [4 * g + i for i in range(4)]
        xtc = np.ascontiguousarray(x[b].T).astype(NPBF16)  # [DM, S]
        cols = []
        for pair in range(2):
            for h in heads[2 * pair : 2 * pair + 2]:
                cols.append(Wq[h * 64 : (h + 1) * 64][perm_l])
        for pair in range(2):
            for h in heads[2 * pair : 2 * pair + 2]:
                cols.append(Wk[h * 64 : (h + 1) * 64][perm_l])
        wqk_np = np.ascontiguousarray(np.concatenate(cols, axis=0).T).astype(NPBF16)
        wv_np = np.ascontiguousarray(
            np.concatenate([Wv[h * 64 : (h + 1) * 64] for h in heads], axis=0).T
        ).astype(NPBF16)
        wo_np = np.ascontiguousarray(
            np.concatenate([Wout[:, h * 64 : (h + 1) * 64].T for h in heads], axis=0)
        ).astype(NPBF16)
        bqk_np = np.zeros((128, 4), dtype=np.float32)
        for ft in range(4):
            bsrc = bq if ft < 2 else bk
            h0 = heads[2 * (ft % 2)]
            h1 = heads[2 * (ft % 2) + 1]
            bqk_np[0:64, ft] = bsrc[h0 * 64 : (h0 + 1) * 64][perm_l]
            bqk_np[64:128, ft] = bsrc[h1 * 64 : (h1 + 1) * 64][perm_l]
        bv_np = np.concatenate(
            [bv_full[h * 64 : (h + 1) * 64] for h in heads]
        ).astype(np.float32)[None, :]
        in_maps.append(
            {
                "xt": xtc,
                "wqk": wqk_np,
                "wv": wv_np,
                "wo": wo_np,
                "bqk": bqk_np,
                "bv": bv_np,
                "ones2": ones2_np,
                "csb": csb_np,
                "snb": snb_np,
                "qks": qks_np,
            }
        )
    return in_maps


def kernel(x, Wqkv, bqkv, q_scale, k_scale, Wout, bout, _trace=False, _results=None):
    x = np.asarray(x, np.float32)
    Wqkv = np.asarray(Wqkv, np.float32)
    bqkv = np.asarray(bqkv, np.float32)
    Wout = np.asarray(Wout, np.float32)
    bout = np.asarray(bout, np.float32)

    nc = _get_program()
    in_maps = _host_inputs(x, Wqkv, bqkv, q_scale, k_scale, Wout, bout)
    res = bass_utils.run_bass_kernel_spmd(
        nc, in_maps, core_ids=list(range(8)), trace=_trace
    )
    if _results is not None:
        _results.append(res)

    out = np.zeros((B, S, DM), dtype=np.float32)
    for c in range(8):
        b = c // 4
        out[b] += res.results[c]["outp"]
    out += bout[None, None, :]
    return out

